# revision 12
# baseline (speedup 1.0000x reference)
"""Trainium2 Bass kernel for the per-cell-MLP "MAR one-sided missingness" model.

Model (per cell (n,t) of a 1024x128 grid):
    xc     = X[n, col_idx[n,t]]
    h      = relu(W_in[n,t,:,0]*xc + W_in[n,t,:,1]*X[n,t] + b_in[n,t,:])   # [H]
    out    = sigmoid(dot(W_out[n,t,:], h) + b_out[n,t])

Sharding: rows N split across 8 cores (128 rows each), fully data parallel.

The kernel is HBM-bandwidth bound: the four per-cell weight tensors dominate
traffic, so they stream as float16 (rel-err ~1e-2 vs the 2e-2 gate, checked
empirically), halving bytes vs f32: 16 MB/core. The neighbor gather
X[n, col_idx[n,t]] is a pure indexing operation and is staged on the host
(like the baseline's host-built one-hot masks, minus the on-device matmul).

Per-core layout: partition dim = t, free dims = (h, n) with n innermost so
the per-cell scalars x[t,n], xc[t,n] broadcast over h via a stride-0 MIDDLE
AP dim — keeping every DVE tensor_tensor in 2x_1p f16 mode (the mode check
only looks at the innermost dim). Streaming over h in blocks of HB=16:

  DVE : m1 = w1 * x_bc          (f16 TT, 2x)
  DVE : a0 = w0 * xc_bc         (f16 TT, 2x)
  PE  : u  = I@m1 + I@a0 + I@b  (identity matmuls accumulate in PSUM f32)
  ACT : ru = relu(u)            (PSUM->SBUF copy with fused ReLU, f16 out)
  DVE : r  = ru * wo            (f16 TT, 2x)
  Pool/DVE: fold r over h by contiguous halves (h outer => halves are
            contiguous slices), last fold + block-accumulate in f32.
Epilogue: out = sigmoid(acc + b_out^T), DMA out, host transposes back.
"""

import numpy as np

N, T, H = 1024, 128, 128
M = 8            # cores
NR = N // M      # rows per core
HB = 16          # h-block size
NB = H // HB     # 8 h-blocks
FD = HB * NR     # free elems per block

_cache = {}


def _build():
    if "nc" in _cache:
        return _cache["nc"]
    import concourse.bacc as bacc
    import concourse.mybir as mybir
    import concourse.tile as tile

    f32 = mybir.dt.float32
    f16 = mybir.dt.float16
    Alu = mybir.AluOpType
    Act = mybir.ActivationFunctionType

    nc = bacc.Bacc()
    # all four weight streams interleaved per h-block:
    # [T, NB, 4, HB*NR] with order (w1, w0, bb, wo) on axis 2
    wall = nc.declare_dram_parameter("wall", [T, NB * 4 * FD], f16, isOutput=False)
    xt = nc.declare_dram_parameter("xt", [T, NR], f16, isOutput=False)
    xct = nc.declare_dram_parameter("xct", [T, NR], f16, isOutput=False)
    bout = nc.declare_dram_parameter("bout", [T, NR], f32, isOutput=False)
    ident = nc.declare_dram_parameter("ident", [128, 128], f16, isOutput=False)
    out = nc.declare_dram_parameter("out", [T, NR], f32, isOutput=True)

    with tile.TileContext(nc) as tc:
        with (
            tc.tile_pool(name="const", bufs=1) as constp,
            tc.tile_pool(name="w", bufs=3) as wp,
            tc.tile_pool(name="mid", bufs=3) as midp,
            tc.tile_pool(name="ps", bufs=2, space="PSUM") as psp,
        ):
            xt_sb = constp.tile([T, NR], f16)
            nc.scalar.dma_start(xt_sb[:], xt[:])
            xct_sb = constp.tile([T, NR], f16)
            nc.scalar.dma_start(xct_sb[:], xct[:])
            bo_sb = constp.tile([T, NR], f32)
            nc.scalar.dma_start(bo_sb[:], bout[:])
            id_sb = constp.tile([128, 128], f16)
            nc.scalar.dma_start(id_sb[:], ident[:])
            acc = constp.tile([T, NR], f32)

            xb = (
                xt_sb[:].rearrange("p (o n) -> p o n", o=1).broadcast_to([T, HB, NR])
            )
            xcb = (
                xct_sb[:].rearrange("p (o n) -> p o n", o=1).broadcast_to([T, HB, NR])
            )

            state = {}

            def front(s):
                c0 = s * 4 * FD
                wt = wp.tile([T, 4 * FD], f16, tag="w")
                if s == 0:
                    # split so m1(0) can start after the first quarter lands
                    for j in range(4):
                        nc.sync.dma_start(
                            wt[:, j * FD : (j + 1) * FD],
                            wall[:, c0 + j * FD : c0 + (j + 1) * FD],
                        )
                else:
                    nc.sync.dma_start(wt[:], wall[:, c0 : c0 + 4 * FD])
                m1 = midp.tile([T, FD], f16, tag="m1")
                nc.vector.tensor_tensor(
                    m1[:].rearrange("p (h n) -> p h n", h=HB),
                    wt[:, 0:FD].rearrange("p (h n) -> p h n", h=HB),
                    xb,
                    Alu.mult,
                )
                a0 = midp.tile([T, FD], f16, tag="a0")
                nc.vector.tensor_tensor(
                    a0[:].rearrange("p (h n) -> p h n", h=HB),
                    wt[:, FD : 2 * FD].rearrange("p (h n) -> p h n", h=HB),
                    xcb,
                    Alu.mult,
                )

                ups = psp.tile([T, FD], f32, tag="u")
                for q in range(FD // 512):
                    qs = slice(q * 512, (q + 1) * 512)
                    nc.tensor.matmul(
                        ups[:, qs], id_sb[:], m1[:, qs], start=True, stop=False
                    )
                    nc.tensor.matmul(
                        ups[:, qs], id_sb[:], a0[:, qs], start=False, stop=False
                    )
                    nc.tensor.matmul(
                        ups[:, qs],
                        id_sb[:],
                        wt[:, 2 * FD + q * 512 : 2 * FD + (q + 1) * 512],
                        start=False,
                        stop=True,
                    )

                ru = midp.tile([T, FD], f16, tag="ru")
                nc.scalar.activation(ru[:], ups[:], Act.Relu)
                state[s] = (ru, wt)

            def mid(s):
                ru, wt = state.pop(s)
                r = midp.tile([T, FD], f16, tag="r")
                nc.vector.tensor_tensor(r[:], ru[:], wt[:, 3 * FD : 4 * FD], Alu.mult)

                # reduce over h: halves are contiguous since h is the outer
                # free dim. Two biggest folds on Pool, rest on DVE (back).
                f1 = midp.tile([T, FD // 2], f16, tag="f1")
                nc.gpsimd.tensor_tensor(f1[:], r[:, : FD // 2], r[:, FD // 2 :], Alu.add)
                f2 = midp.tile([T, FD // 4], f16, tag="f2")
                nc.gpsimd.tensor_tensor(f2[:], f1[:, : FD // 4], f1[:, FD // 4 :], Alu.add)
                state[("f", s)] = f2

            def back(s):
                f2 = state.pop(("f", s))
                f3 = midp.tile([T, FD // 8], f16, tag="f3")
                nc.vector.tensor_tensor(f3[:], f2[:, : FD // 8], f2[:, FD // 8 :], Alu.add)
                if s == 0:
                    nc.vector.tensor_tensor(acc[:], f3[:, :NR], f3[:, NR:], Alu.add)
                else:
                    rb = midp.tile([T, NR], f32, tag="rb")
                    nc.vector.tensor_tensor(rb[:], f3[:, :NR], f3[:, NR:], Alu.add)
                    nc.vector.tensor_tensor(acc[:], acc[:], rb[:], Alu.add)

            for stage in range(NB + 3):
                if stage < NB:
                    front(stage)
                if 1 <= stage < NB + 1:
                    mid(stage - 1)
                if 3 <= stage < NB + 3:
                    back(stage - 3)

            lg = midp.tile([T, NR], f32, tag="lg")
            nc.vector.tensor_tensor(lg[:], acc[:], bo_sb[:], Alu.add)
            ot = midp.tile([T, NR], f32, tag="ot")
            nc.scalar.activation(ot[:], lg[:], Act.Sigmoid)
            nc.sync.dma_start(out[:], ot[:])

    nc.compile()
    _cache["nc"] = nc
    return nc


def make_in_maps(X, W_in, b_in, W_out, b_out, col_idx):
    X = np.asarray(X, dtype=np.float32)
    W_in = np.asarray(W_in, dtype=np.float32)
    b_in = np.asarray(b_in, dtype=np.float32)
    W_out = np.asarray(W_out, dtype=np.float32)
    b_out = np.asarray(b_out, dtype=np.float32)
    col_idx = np.asarray(col_idx)

    xc = np.take_along_axis(X, col_idx, axis=1)  # [N, T] neighbor gather
    ident = np.eye(128, dtype=np.float16)

    w0g = W_in[:, :, :, 0].astype(np.float16)  # [N, T, H]
    w1g = W_in[:, :, :, 1].astype(np.float16)
    bbg = b_in.astype(np.float16)
    wog = W_out.astype(np.float16)

    in_maps = []
    for c in range(M):
        sl = slice(c * NR, (c + 1) * NR)

        def t_hn(a):  # [NR, T, H] -> [T, NB, HB, NR] f16
            return np.ascontiguousarray(a[sl].transpose(1, 2, 0)).reshape(
                T, NB, HB, NR
            )

        # interleave (w1, w0, bb, wo) per h-block: [T, NB, 4, HB, NR]
        wall = np.stack(
            [t_hn(w1g), t_hn(w0g), t_hn(bbg), t_hn(wog)], axis=2
        ).reshape(T, NB * 4 * HB * NR)

        in_maps.append(
            {
                "wall": np.ascontiguousarray(wall),
                "xt": np.ascontiguousarray(X[sl].T.astype(np.float16)),
                "xct": np.ascontiguousarray(xc[sl].T.astype(np.float16)),
                "bout": np.ascontiguousarray(b_out[sl].T),
                "ident": ident,
            }
        )
    return in_maps


def kernel(X, W_in, b_in, W_out, b_out, col_idx):
    from concourse.bass_utils import run_bass_kernel_spmd

    nc = _build()
    in_maps = make_in_maps(X, W_in, b_in, W_out, b_out, col_idx)
    res = run_bass_kernel_spmd(nc, in_maps, list(range(M))).results
    out = np.empty((N, T), np.float32)
    for c in range(M):
        out[c * NR : (c + 1) * NR] = res[c]["out"].T
    return out
